# revision 9
# baseline (speedup 1.0000x reference)
"""SAGAN-style self-attention block on 8 trn2 NeuronCores.

Sharding: core = (b, half) with b = core // 2 (batch), half = core % 2
(query-row half of the image).  Each core gets x[b] as bf16 [128, 8192]:
partitions 0-63 = own 64 image rows (queries + residual), 64-127 = the
other half (needed only for pooled keys/values phi/g).  Pure SPMD.

Per-core dataflow (C=64, NH=8192 queries, M=4096 pooled keys):
  conv:  per 512-col slice, three concurrent PE tiles: own g+phi conv
         (rows 0-63 -> cols 0-39), other-half g+phi (rows 64-127), and
         theta (rows 0-63 -> cols 64-71 via col-group tiling).  Theta is
         copied out on ScalarE; 2x2 maxpool = reduce_max (horizontal,
         straight from PSUM) + tensor_max (vertical, SBUF bf16); g
         chunks transposed on PE into gt with a ones column appended
         (col 32 of each 33-wide block) for the softmax denominator.
  attn:  per 512-query block, 11 chunk-groups (3,3,...,3,2).  Scores:
         K=8 bf16 matmuls on 3 concurrent 32-row PE strips, one PSUM
         bank per 128-key chunk (two 3-bank buffers alternate so the PE
         computes group g+1 while ScalarE exps group g).  exp on
         ScalarE (PSUM f32 -> SBUF bf16).  o_mid: chunk PAIRS on two
         concurrent 33-col PE strips (cols 0-32 / 64-96) accumulating
         po[0:33] (even chunks) and po[64:97] (odd) in one PSUM bank;
         each strip's first matmul carries start=True (the has_written
         clear is column-scoped).
  tail:  merge lo+hi halves (copy + partition-move DMA + add),
         reciprocal of the denominator row on VectorE (bf16),
         gamma folded into w_o on the host, 1/denom broadcast via a
         K=1 matmul, normalize, output conv, residual add from bf16 x.
         The tail of block nb is emitted interleaved into block nb+1 so
         its PE matmuls never stall the score pipeline.
"""

import numpy as np

_CACHE = {}

C = 64
NH = 8192  # queries per core
M = 4096  # pooled key positions
NB = 16  # 512-query blocks
MCH = 32  # 128-wide m-chunks
GROUPS = [3] * 10 + [2]  # score chunk-groups per block (sum = 32)


def _split_multiwaits(nc):
    """This walrus build accepts only one sync-wait per instruction;
    hoist extras onto NoOp carriers on the same engine."""
    import concourse.mybir as mybir

    for f in nc.m.functions:
        for bb in f.blocks:
            out = []
            changed = False
            for ins in bb.instructions:
                si = getattr(ins, "sync_info", None)
                conds = list(si.on_wait) if si is not None and si.on_wait else []
                if len(conds) > 1:
                    for c in conds[:-1]:
                        es = mybir.InstNoOp(
                            name=nc.get_next_instruction_name(), ins=[], outs=[]
                        )
                        es.engine = ins.engine
                        es.sync_info = mybir.SyncInfo(on_wait=[c], on_update=[])
                        nc.register_instruction(es, overwrite=True)
                        out.append(es)
                    si.on_wait = [conds[-1]]
                    changed = True
                out.append(ins)
            if changed:
                bb.instructions = out


def _build():
    import concourse.bass as bass
    import concourse.mybir as mybir
    from concourse import tile

    f32 = mybir.dt.float32
    bf16 = mybir.dt.bfloat16
    Exp = mybir.ActivationFunctionType.Exp

    nc = bass.Bass()
    xb_d = nc.dram_tensor("xb", [128, NH], bf16, kind="ExternalInput")
    wall_d = nc.dram_tensor("wall", [128, 48], bf16, kind="ExternalInput")
    wot_d = nc.dram_tensor("wot", [32, 64], bf16, kind="ExternalInput")
    id_d = nc.dram_tensor("ident", [32, 32], bf16, kind="ExternalInput")
    out_d = nc.dram_tensor("out", [128, NH // 2], f32, kind="ExternalOutput")

    with tile.TileContext(nc) as tc:
        with (
            tc.tile_pool(name="consts", bufs=1) as cpool,
            tc.tile_pool(name="xin", bufs=8) as xpool,
            tc.tile_pool(name="big", bufs=1) as bpool,
        ):
            wall_sb = cpool.tile([128, 48], bf16, tag="wall")
            wot_sb = cpool.tile([32, 64], bf16, tag="wot")
            id_sb = cpool.tile([32, 32], bf16, tag="ident")
            ones1 = cpool.tile([33, 96], bf16, tag="ones1")

            xl = [
                xpool.tile([128, 1024], bf16, tag="xl", name=f"xl{i}")
                for i in range(8)
            ]

            th = bpool.tile([72, NH], bf16, tag="theta")
            ph = bpool.tile([72, M], bf16, tag="phi")
            gp = bpool.tile([40, M], bf16, tag="gphi")
            gt = bpool.tile([128, 33 * MCH], bf16, tag="gt")

            nc.sync.dma_start(out=wall_sb[:], in_=wall_d[:])
            nc.sync.dma_start(out=wot_sb[:], in_=wot_d[:])
            nc.sync.dma_start(out=id_sb[:], in_=id_d[:])
            for i in range(8):
                nc.sync.dma_start(out=xl[i][:], in_=xb_d[:, 1024 * i : 1024 * (i + 1)])
            nc.vector.memset(ones1[:], 1.0)
            nc.vector.memset(gt[:], 1.0)

            # ---- conv + pool + g-transpose phase -------------------------
            with (
                tc.tile_pool(name="cpa", bufs=2, space="PSUM") as cpa,
                tc.tile_pool(name="cpb", bufs=2, space="PSUM") as cpb,
                tc.tile_pool(name="cpt", bufs=2, space="PSUM") as cpt,
                tc.tile_pool(name="tps", bufs=2, space="PSUM") as tps,
                tc.tile_pool(name="scr", bufs=3) as scr,
            ):

                def pool40(psrc, moff):
                    # [40, 512] PSUM (4 image rows x 128 cols) -> [40, 128]
                    # horizontal 2:1 straight from PSUM, vertical on SBUF.
                    s1 = scr.tile([40, 256], bf16, tag="s1")
                    v = psrc[0:40, :].rearrange("p (x two) -> p x two", two=2)
                    nc.vector.reduce_max(s1[:], v, axis=mybir.AxisListType.X)
                    u = s1[:].rearrange("p (r two c) -> p r two c", two=2, c=64)
                    nc.vector.tensor_max(
                        gp[0:40, moff : moff + 128].rearrange(
                            "p (r c) -> p r c", c=64
                        ),
                        u[:, :, 0, :],
                        u[:, :, 1, :],
                    )

                def transp(mc):
                    pt = tps.tile([128, 32], bf16, tag="gtp")
                    nc.tensor.transpose(
                        pt[:], gp[0:32, 128 * mc : 128 * mc + 128], id_sb[:]
                    )
                    nc.scalar.copy(gt[:, 33 * mc : 33 * mc + 32], pt[:])

                for t in range(16):
                    xi, xo = t // 2, (t % 2) * 512
                    rhs_own = xl[xi][0:64, xo : xo + 512]
                    rhs_oth = xl[xi][64:128, xo : xo + 512]
                    pa = cpa.tile([40, 512], f32, tag="conv")
                    nc.tensor.matmul(
                        pa[:], wall_sb[0:64, 0:40], rhs_own,
                        start=True, stop=True, tile_position=(0, 0),
                    )
                    pt8 = cpt.tile([128, 512], f32, tag="th")
                    nc.tensor.matmul(
                        pt8[64:72, :], wall_sb[0:64, 40:48], rhs_own,
                        start=True, stop=True, tile_position=(0, 64),
                        skip_group_check=True,
                    )
                    pb = cpb.tile([40, 512], f32, tag="conv")
                    nc.tensor.matmul(
                        pb[:], wall_sb[64:128, 0:40], rhs_oth,
                        start=True, stop=True, tile_position=(64, 0),
                    )
                    nc.scalar.copy(th[64:72, 512 * t : 512 * t + 512], pt8[64:72, :])
                    pool40(pa, 128 * t)
                    pool40(pb, 2048 + 128 * t)
                    transp(t)
                    transp(16 + t)

            # replicate theta/phi across the PE row strips (0-7/32-39/64-71)
            nc.sync.dma_start(out=th[0:8, :], in_=th[64:72, :])
            nc.sync.dma_start(out=th[32:40, :], in_=th[64:72, :])
            nc.sync.dma_start(out=ph[0:8, :], in_=gp[32:40, :])
            nc.sync.dma_start(out=ph[32:40, :], in_=gp[32:40, :])
            nc.sync.dma_start(out=ph[64:72, :], in_=gp[32:40, :])

            # ---- attention phase ----------------------------------------
            with (
                tc.tile_pool(name="psA", bufs=1, space="PSUM") as psA,
                tc.tile_pool(name="psB", bufs=1, space="PSUM") as psB,
                tc.tile_pool(name="pop", bufs=1, space="PSUM") as pop,
                tc.tile_pool(name="tbp", bufs=1, space="PSUM") as tbp,
                tc.tile_pool(name="ep", bufs=4) as ep,
                tc.tile_pool(name="oms", bufs=2) as oms,
                tc.tile_pool(name="hip", bufs=2) as hip,
                tc.tile_pool(name="rcs", bufs=2) as rcs,
                tc.tile_pool(name="omns", bufs=2) as omns,
                tc.tile_pool(name="stg", bufs=3) as stg,
            ):
                # per-block pipeline state
                st = {}  # nb -> dict(po, et_sl, next_pair, om)

                def emit_scores_exp(nb, gi, parity):
                    if nb not in st:
                        st[nb] = {"et_sl": {}, "next_pair": 0, "c0": 0}
                    s = st[nb]
                    gsz = GROUPS[gi]
                    c0 = s["c0"]
                    pool = psA if parity % 2 == 0 else psB
                    ps = pool.tile([128, 1536], f32, tag="ps")
                    for j in range(gsz):
                        mc = c0 + j
                        nc.tensor.matmul(
                            ps[:, 512 * j : 512 * j + 512],
                            ph[32 * j : 32 * j + 8, 128 * mc : 128 * mc + 128],
                            th[32 * j : 32 * j + 8, 512 * nb : 512 * nb + 512],
                            start=True, stop=True, tile_position=(32 * j, 0),
                        )
                    et = ep.tile([128, 1536], bf16, tag="et")
                    nc.scalar.activation(
                        et[:, 0 : 512 * gsz], ps[:, 0 : 512 * gsz], Exp
                    )
                    for j in range(gsz):
                        s["et_sl"][c0 + j] = (et, 512 * j)
                    s["c0"] = c0 + gsz

                def emit_pairs(nb, through_chunks):
                    s = st[nb]
                    if "po" not in s:
                        s["po"] = pop.tile([128, 512], f32, tag="po", name=f"po{nb}")
                    po = s["po"]
                    while 2 * s["next_pair"] + 1 < through_chunks:
                        p = s["next_pair"]
                        a, b = 2 * p, 2 * p + 1
                        ta, oa = s["et_sl"][a]
                        tb_, ob = s["et_sl"][b]
                        nc.tensor.matmul(
                            po[0:33, :],
                            gt[:, 33 * a : 33 * a + 33],
                            ta[:, oa : oa + 512],
                            start=(p == 0), stop=(p == 15),
                            tile_position=(0, 0), skip_group_check=True,
                        )
                        nc.tensor.matmul(
                            po[64:97, :],
                            gt[:, 33 * b : 33 * b + 33],
                            tb_[:, ob : ob + 512],
                            start=(p == 0), stop=(p == 15),
                            tile_position=(0, 64), skip_group_check=True,
                        )
                        s["next_pair"] = p + 1

                def early_tail(nb):
                    # merge lo+hi halves of po; frees the po bank promptly
                    s = st[nb]
                    hi = hip.tile([128, 512], f32, tag="hi")
                    nc.vector.tensor_copy(hi[64:97, :], s["po"][64:97, :])
                    hig = hip.tile([33, 512], f32, tag="hig")
                    nc.sync.dma_start(out=hig[:], in_=hi[64:97, :])
                    om = oms.tile([33, 512], f32, tag="om", name=f"om{nb}")
                    nc.vector.tensor_add(om[:], s["po"][0:33, :], hig[:])
                    s["om"] = om

                def late_tail(nb):
                    om = st[nb]["om"]
                    rc = rcs.tile([33, 512], bf16, tag="rc")
                    with nc.allow_low_precision(
                        reason="softmax 1/denom in bf16; output rel-err budget 2e-2"
                    ):
                        nc.vector.reciprocal(rc[32:33, :], om[32:33, :])
                    tb = tbp.tile([128, 512], f32, tag="tb")
                    nc.tensor.matmul(
                        tb[0:32, :], ones1[32:33, 0:32], rc[32:33, :],
                        start=True, stop=True, tile_position=(32, 0),
                        skip_group_check=True,
                    )
                    omn = omns.tile([32, 512], bf16, tag="omn")
                    nc.vector.tensor_mul(omn[:], om[0:32, :], tb[0:32, :])
                    nc.tensor.matmul(
                        tb[0:64, :], wot_sb[:], omn[:],
                        start=True, stop=True, tile_position=(0, 0),
                        skip_group_check=True,
                    )
                    stage = stg.tile([64, 512], f32, tag="stage")
                    nc.vector.tensor_add(
                        stage[:],
                        tb[0:64, :],
                        xl[nb // 2][0:64, (nb % 2) * 512 : (nb % 2) * 512 + 512],
                    )
                    pp = 0 if nb < 8 else 64
                    off = 512 * nb if nb < 8 else 512 * (nb - 8)
                    nc.sync.dma_start(
                        out=out_d[pp : pp + 64, off : off + 512], in_=stage[:]
                    )
                    del st[nb]

                def chunks_done(gi):
                    return 32 if gi >= 10 else 3 * (gi + 1)

                # pairs lag TWO slots behind their exp so score bursts
                # never queue behind pair matmuls that still wait on exp
                slots = [(nb, gi) for nb in range(NB) for gi in range(len(GROUPS))]

                def lagged(k):
                    pnb, pgi = slots[k]
                    emit_pairs(pnb, chunks_done(pgi))
                    if pgi == len(GROUPS) - 1:
                        early_tail(pnb)

                for k, (nb, gi) in enumerate(slots):
                    emit_scores_exp(nb, gi, k)
                    if k >= 2:
                        lagged(k - 2)
                    if gi == 4 and nb >= 1:
                        late_tail(nb - 1)
                lagged(len(slots) - 2)
                lagged(len(slots) - 1)
                late_tail(NB - 1)

    _split_multiwaits(nc)
    return nc


def _get_program():
    if "nc" not in _CACHE:
        _CACHE["nc"] = _build()
    return _CACHE["nc"]


def _make_in_maps(x, w_theta, w_phi, w_g, w_o, gamma):
    import ml_dtypes

    bf16 = ml_dtypes.bfloat16
    x = np.asarray(x, np.float32)
    w_theta = np.asarray(w_theta, np.float32)
    w_phi = np.asarray(w_phi, np.float32)
    w_g = np.asarray(w_g, np.float32)
    w_o = np.asarray(w_o, np.float32)
    B, C_, H, W = x.shape
    # conv weight column layout: [g(32) | phi(8) | theta(8)]
    w_all = np.concatenate([w_g.T, w_phi.T, w_theta.T], axis=1)  # [64, 48]
    wall2 = np.ascontiguousarray(
        np.concatenate([w_all, w_all], axis=0)
    ).astype(bf16)
    wot = np.ascontiguousarray(float(gamma) * w_o.T).astype(bf16)  # [32, 64]
    ident = np.eye(32, dtype=np.float32).astype(bf16)
    xb = x.astype(bf16)
    in_maps = []
    for core in range(8):
        b, half = core // 2, core % 2
        xbb = xb[b].reshape(C_, H, W)
        xo = xbb[:, 64 * half : 64 * half + 64, :].reshape(C_, NH)
        xr = xbb[:, 64 * (1 - half) : 64 * (1 - half) + 64, :].reshape(C_, NH)
        xlc = np.ascontiguousarray(np.concatenate([xo, xr], axis=0))
        in_maps.append({"xb": xlc, "wall": wall2, "wot": wot, "ident": ident})
    return in_maps


def _assemble(results, B, C_, H, W):
    out = np.zeros((B, C_, H, W), np.float32)
    for core in range(8):
        b, half = core // 2, core % 2
        o = np.asarray(results[core]["out"])  # [128, 4096]
        oh = np.concatenate([o[0:64, :], o[64:128, :]], axis=1)  # [64, 8192]
        out[b, :, 64 * half : 64 * half + 64, :] = oh.reshape(C_, 64, W)
    return out


def kernel(x, w_theta, w_phi, w_g, w_o, gamma, _trace=False):
    from concourse.bass_utils import run_bass_kernel_spmd

    x = np.asarray(x, np.float32)
    nc = _get_program()
    in_maps = _make_in_maps(x, w_theta, w_phi, w_g, w_o, gamma)
    res = run_bass_kernel_spmd(nc, in_maps, list(range(8)), trace=_trace)
    out = _assemble(res.results, *x.shape)
    if _trace:
        kernel._last_result = res
    return out


# revision 10
# speedup vs baseline: 1.0833x; 1.0833x over previous
"""SAGAN-style self-attention block on 8 trn2 NeuronCores.

Sharding: core = (b, half) with b = core // 2 (batch), half = core % 2
(query-row half of the image).  Each core gets x[b] as bf16 [128, 8192]:
partitions 0-63 = own 64 image rows (queries + residual), 64-127 = the
other half (needed only for pooled keys/values phi/g).  Pure SPMD.

Per-core dataflow (C=64, NH=8192 queries, M=4096 pooled keys):
  conv:  per 512-col slice, three concurrent PE tiles: own g+phi conv
         (rows 0-63 -> cols 0-39), other-half g+phi (rows 64-127), and
         theta (rows 0-63 -> cols 64-71 via col-group tiling).  Theta is
         copied out on ScalarE; 2x2 maxpool = reduce_max (horizontal,
         straight from PSUM) + tensor_max (vertical, SBUF bf16); g
         chunks transposed on PE into gt with a ones column appended
         (col 32 of each 33-wide block) for the softmax denominator.
  attn:  per 512-query block, 11 chunk-groups (3,3,...,3,2).  Scores:
         K=8 bf16 matmuls on 3 concurrent 32-row PE strips, one PSUM
         bank per 128-key chunk (two 3-bank buffers alternate so the PE
         computes group g+1 while ScalarE exps group g).  exp on
         ScalarE (PSUM f32 -> SBUF bf16).  o_mid: chunk PAIRS on two
         concurrent 33-col PE strips (cols 0-32 / 64-96) accumulating
         po[0:33] (even chunks) and po[64:97] (odd) in one PSUM bank;
         each strip's first matmul carries start=True (the has_written
         clear is column-scoped).
  tail:  merge lo+hi halves (copy + partition-move DMA + add),
         reciprocal of the denominator row on VectorE (bf16),
         gamma folded into w_o on the host, 1/denom broadcast via a
         K=1 matmul, normalize, output conv, residual add from bf16 x.
         The tail of block nb is emitted interleaved into block nb+1 so
         its PE matmuls never stall the score pipeline.
"""

import numpy as np

_CACHE = {}

C = 64
NH = 8192  # queries per core
M = 4096  # pooled key positions
NB = 16  # 512-query blocks
MCH = 32  # 128-wide m-chunks
GROUPS = [3] * 10 + [2]  # score chunk-groups per block (sum = 32)


def _split_multiwaits(nc):
    """This walrus build accepts only one sync-wait per instruction;
    hoist extras onto NoOp carriers on the same engine."""
    import concourse.mybir as mybir

    for f in nc.m.functions:
        for bb in f.blocks:
            out = []
            changed = False
            for ins in bb.instructions:
                si = getattr(ins, "sync_info", None)
                conds = list(si.on_wait) if si is not None and si.on_wait else []
                if len(conds) > 1:
                    for c in conds[:-1]:
                        es = mybir.InstNoOp(
                            name=nc.get_next_instruction_name(), ins=[], outs=[]
                        )
                        es.engine = ins.engine
                        es.sync_info = mybir.SyncInfo(on_wait=[c], on_update=[])
                        nc.register_instruction(es, overwrite=True)
                        out.append(es)
                    si.on_wait = [conds[-1]]
                    changed = True
                out.append(ins)
            if changed:
                bb.instructions = out


def _build():
    import concourse.bass as bass
    import concourse.mybir as mybir
    from concourse import tile

    f32 = mybir.dt.float32
    bf16 = mybir.dt.bfloat16
    Exp = mybir.ActivationFunctionType.Exp

    nc = bass.Bass()
    xb_d = nc.dram_tensor("xb", [128, NH], bf16, kind="ExternalInput")
    wall_d = nc.dram_tensor("wall", [128, 48], bf16, kind="ExternalInput")
    wot_d = nc.dram_tensor("wot", [32, 64], bf16, kind="ExternalInput")
    id_d = nc.dram_tensor("ident", [32, 32], bf16, kind="ExternalInput")
    out_d = nc.dram_tensor("out", [128, NH // 2], f32, kind="ExternalOutput")

    with tile.TileContext(nc) as tc:
        with (
            tc.tile_pool(name="consts", bufs=1) as cpool,
            tc.tile_pool(name="xin", bufs=8) as xpool,
            tc.tile_pool(name="big", bufs=1) as bpool,
        ):
            wall_sb = cpool.tile([128, 48], bf16, tag="wall")
            wot_sb = cpool.tile([32, 64], bf16, tag="wot")
            id_sb = cpool.tile([32, 32], bf16, tag="ident")
            ones1 = cpool.tile([33, 96], bf16, tag="ones1")

            xl = [
                xpool.tile([128, 1024], bf16, tag="xl", name=f"xl{i}")
                for i in range(8)
            ]

            th = bpool.tile([72, NH], bf16, tag="theta")
            ph = bpool.tile([72, M], bf16, tag="phi")
            gp = bpool.tile([40, M], bf16, tag="gphi")
            gt = bpool.tile([128, 33 * MCH], bf16, tag="gt")

            nc.sync.dma_start(out=wall_sb[:], in_=wall_d[:])
            nc.sync.dma_start(out=wot_sb[:], in_=wot_d[:])
            nc.sync.dma_start(out=id_sb[:], in_=id_d[:])
            for i in range(8):
                nc.sync.dma_start(out=xl[i][:], in_=xb_d[:, 1024 * i : 1024 * (i + 1)])
            nc.vector.memset(ones1[:], 1.0)
            nc.vector.memset(gt[:], 1.0)

            # ---- conv + pool + g-transpose phase -------------------------
            with (
                tc.tile_pool(name="cpa", bufs=2, space="PSUM") as cpa,
                tc.tile_pool(name="cpb", bufs=2, space="PSUM") as cpb,
                tc.tile_pool(name="cpt", bufs=2, space="PSUM") as cpt,
                tc.tile_pool(name="tps", bufs=2, space="PSUM") as tps,
                tc.tile_pool(name="scr", bufs=3) as scr,
            ):

                def pool40(psrc, moff):
                    # [40, 512] PSUM (4 image rows x 128 cols) -> [40, 128]
                    # horizontal 2:1 straight from PSUM, vertical on SBUF.
                    s1 = scr.tile([40, 256], bf16, tag="s1")
                    v = psrc[0:40, :].rearrange("p (x two) -> p x two", two=2)
                    nc.vector.reduce_max(s1[:], v, axis=mybir.AxisListType.X)
                    u = s1[:].rearrange("p (r two c) -> p r two c", two=2, c=64)
                    nc.vector.tensor_max(
                        gp[0:40, moff : moff + 128].rearrange(
                            "p (r c) -> p r c", c=64
                        ),
                        u[:, :, 0, :],
                        u[:, :, 1, :],
                    )

                def transp(mc):
                    pt = tps.tile([128, 32], bf16, tag="gtp")
                    nc.tensor.transpose(
                        pt[:], gp[0:32, 128 * mc : 128 * mc + 128], id_sb[:]
                    )
                    nc.scalar.copy(gt[:, 33 * mc : 33 * mc + 32], pt[:])

                for t in range(16):
                    xi, xo = t // 2, (t % 2) * 512
                    rhs_own = xl[xi][0:64, xo : xo + 512]
                    rhs_oth = xl[xi][64:128, xo : xo + 512]
                    pa = cpa.tile([40, 512], f32, tag="conv")
                    nc.tensor.matmul(
                        pa[:], wall_sb[0:64, 0:40], rhs_own,
                        start=True, stop=True, tile_position=(0, 0),
                    )
                    pt8 = cpt.tile([128, 512], f32, tag="th")
                    nc.tensor.matmul(
                        pt8[64:72, :], wall_sb[0:64, 40:48], rhs_own,
                        start=True, stop=True, tile_position=(0, 64),
                        skip_group_check=True,
                    )
                    pb = cpb.tile([40, 512], f32, tag="conv")
                    nc.tensor.matmul(
                        pb[:], wall_sb[64:128, 0:40], rhs_oth,
                        start=True, stop=True, tile_position=(64, 0),
                    )
                    nc.scalar.copy(th[64:72, 512 * t : 512 * t + 512], pt8[64:72, :])
                    pool40(pa, 128 * t)
                    pool40(pb, 2048 + 128 * t)
                    transp(t)
                    transp(16 + t)

            # replicate theta/phi across the PE row strips (0-7/32-39/64-71)
            nc.sync.dma_start(out=th[0:8, :], in_=th[64:72, :])
            nc.sync.dma_start(out=th[32:40, :], in_=th[64:72, :])
            nc.sync.dma_start(out=ph[0:8, :], in_=gp[32:40, :])
            nc.sync.dma_start(out=ph[32:40, :], in_=gp[32:40, :])
            nc.sync.dma_start(out=ph[64:72, :], in_=gp[32:40, :])

            # ---- attention phase ----------------------------------------
            with (
                tc.tile_pool(name="psA", bufs=1, space="PSUM") as psA,
                tc.tile_pool(name="psB", bufs=1, space="PSUM") as psB,
                tc.tile_pool(name="pop", bufs=1, space="PSUM") as pop,
                tc.tile_pool(name="tbp", bufs=1, space="PSUM") as tbp,
                tc.tile_pool(name="ep", bufs=4) as ep,
                tc.tile_pool(name="oms", bufs=2) as oms,
                tc.tile_pool(name="hip", bufs=2) as hip,
                tc.tile_pool(name="rcs", bufs=2) as rcs,
                tc.tile_pool(name="omns", bufs=2) as omns,
                tc.tile_pool(name="stg", bufs=3) as stg,
            ):
                # per-block pipeline state
                st = {}  # nb -> dict(po, et_sl, next_pair, om)

                def emit_scores_exp(nb, gi, parity):
                    if nb not in st:
                        st[nb] = {"et_sl": {}, "next_pair": 0, "c0": 0}
                    s = st[nb]
                    gsz = GROUPS[gi]
                    c0 = s["c0"]
                    pool = psA if parity % 2 == 0 else psB
                    ps = pool.tile([128, 1536], f32, tag="ps")
                    for j in range(gsz):
                        mc = c0 + j
                        nc.tensor.matmul(
                            ps[:, 512 * j : 512 * j + 512],
                            ph[32 * j : 32 * j + 8, 128 * mc : 128 * mc + 128],
                            th[32 * j : 32 * j + 8, 512 * nb : 512 * nb + 512],
                            start=True, stop=True, tile_position=(32 * j, 0),
                        )
                    et = ep.tile([128, 1536], bf16, tag="et")
                    nc.scalar.activation(
                        et[:, 0 : 512 * gsz], ps[:, 0 : 512 * gsz], Exp
                    )
                    for j in range(gsz):
                        s["et_sl"][c0 + j] = (et, 512 * j)
                    s["c0"] = c0 + gsz

                def emit_pairs(nb, through_chunks):
                    s = st[nb]
                    if "po" not in s:
                        s["po"] = pop.tile([128, 512], f32, tag="po", name=f"po{nb}")
                    po = s["po"]
                    while 2 * s["next_pair"] + 1 < through_chunks:
                        p = s["next_pair"]
                        a, b = 2 * p, 2 * p + 1
                        ta, oa = s["et_sl"][a]
                        tb_, ob = s["et_sl"][b]
                        nc.tensor.matmul(
                            po[0:33, :],
                            gt[:, 33 * a : 33 * a + 33],
                            ta[:, oa : oa + 512],
                            start=(p == 0), stop=(p == 15),
                            tile_position=(0, 0), skip_group_check=True,
                        )
                        nc.tensor.matmul(
                            po[64:97, :],
                            gt[:, 33 * b : 33 * b + 33],
                            tb_[:, ob : ob + 512],
                            start=(p == 0), stop=(p == 15),
                            tile_position=(0, 64), skip_group_check=True,
                        )
                        s["next_pair"] = p + 1

                def early_tail(nb):
                    # merge lo+hi halves of po; frees the po bank promptly
                    s = st[nb]
                    hi = hip.tile([128, 512], f32, tag="hi")
                    nc.vector.tensor_copy(hi[64:97, :], s["po"][64:97, :])
                    hig = hip.tile([33, 512], f32, tag="hig")
                    nc.sync.dma_start(out=hig[:], in_=hi[64:97, :])
                    om = oms.tile([33, 512], f32, tag="om", name=f"om{nb}")
                    nc.vector.tensor_add(om[:], s["po"][0:33, :], hig[:])
                    s["om"] = om

                def late_tail(nb):
                    om = st[nb]["om"]
                    rc = rcs.tile([33, 512], bf16, tag="rc")
                    with nc.allow_low_precision(
                        reason="softmax 1/denom in bf16; output rel-err budget 2e-2"
                    ):
                        nc.vector.reciprocal(rc[32:33, :], om[32:33, :])
                    tb = tbp.tile([128, 512], f32, tag="tb")
                    nc.tensor.matmul(
                        tb[0:32, :], ones1[32:33, 0:32], rc[32:33, :],
                        start=True, stop=True, tile_position=(32, 0),
                        skip_group_check=True,
                    )
                    omn = omns.tile([32, 512], bf16, tag="omn")
                    nc.vector.tensor_mul(omn[:], om[0:32, :], tb[0:32, :])
                    nc.tensor.matmul(
                        tb[0:64, :], wot_sb[:], omn[:],
                        start=True, stop=True, tile_position=(0, 0),
                        skip_group_check=True,
                    )
                    stage = stg.tile([64, 512], f32, tag="stage")
                    nc.vector.tensor_add(
                        stage[:],
                        tb[0:64, :],
                        xl[nb // 2][0:64, (nb % 2) * 512 : (nb % 2) * 512 + 512],
                    )
                    pp = 0 if nb < 8 else 64
                    off = 512 * nb if nb < 8 else 512 * (nb - 8)
                    nc.sync.dma_start(
                        out=out_d[pp : pp + 64, off : off + 512], in_=stage[:]
                    )
                    del st[nb]

                def chunks_done(gi):
                    return 32 if gi >= 10 else 3 * (gi + 1)

                # pairs lag TWO slots behind their exp so score bursts
                # never queue behind pair matmuls that still wait on exp
                slots = [(nb, gi) for nb in range(NB) for gi in range(len(GROUPS))]

                def lagged(k):
                    pnb, pgi = slots[k]
                    emit_pairs(pnb, chunks_done(pgi))
                    if pgi == len(GROUPS) - 1:
                        early_tail(pnb)

                for k, (nb, gi) in enumerate(slots):
                    emit_scores_exp(nb, gi, k)
                    if k >= 2:
                        lagged(k - 2)
                    if gi == 8 and nb >= 1:
                        late_tail(nb - 1)
                lagged(len(slots) - 2)
                lagged(len(slots) - 1)
                late_tail(NB - 1)

    _split_multiwaits(nc)
    return nc


def _get_program():
    if "nc" not in _CACHE:
        _CACHE["nc"] = _build()
    return _CACHE["nc"]


def _make_in_maps(x, w_theta, w_phi, w_g, w_o, gamma):
    import ml_dtypes

    bf16 = ml_dtypes.bfloat16
    x = np.asarray(x, np.float32)
    w_theta = np.asarray(w_theta, np.float32)
    w_phi = np.asarray(w_phi, np.float32)
    w_g = np.asarray(w_g, np.float32)
    w_o = np.asarray(w_o, np.float32)
    B, C_, H, W = x.shape
    # conv weight column layout: [g(32) | phi(8) | theta(8)]
    w_all = np.concatenate([w_g.T, w_phi.T, w_theta.T], axis=1)  # [64, 48]
    wall2 = np.ascontiguousarray(
        np.concatenate([w_all, w_all], axis=0)
    ).astype(bf16)
    wot = np.ascontiguousarray(float(gamma) * w_o.T).astype(bf16)  # [32, 64]
    ident = np.eye(32, dtype=np.float32).astype(bf16)
    xb = x.astype(bf16)
    in_maps = []
    for core in range(8):
        b, half = core // 2, core % 2
        xbb = xb[b].reshape(C_, H, W)
        xo = xbb[:, 64 * half : 64 * half + 64, :].reshape(C_, NH)
        xr = xbb[:, 64 * (1 - half) : 64 * (1 - half) + 64, :].reshape(C_, NH)
        xlc = np.ascontiguousarray(np.concatenate([xo, xr], axis=0))
        in_maps.append({"xb": xlc, "wall": wall2, "wot": wot, "ident": ident})
    return in_maps


def _assemble(results, B, C_, H, W):
    out = np.zeros((B, C_, H, W), np.float32)
    for core in range(8):
        b, half = core // 2, core % 2
        o = np.asarray(results[core]["out"])  # [128, 4096]
        oh = np.concatenate([o[0:64, :], o[64:128, :]], axis=1)  # [64, 8192]
        out[b, :, 64 * half : 64 * half + 64, :] = oh.reshape(C_, 64, W)
    return out


def kernel(x, w_theta, w_phi, w_g, w_o, gamma, _trace=False):
    from concourse.bass_utils import run_bass_kernel_spmd

    x = np.asarray(x, np.float32)
    nc = _get_program()
    in_maps = _make_in_maps(x, w_theta, w_phi, w_g, w_o, gamma)
    res = run_bass_kernel_spmd(nc, in_maps, list(range(8)), trace=_trace)
    out = _assemble(res.results, *x.shape)
    if _trace:
        kernel._last_result = res
    return out
